# revision 28
# baseline (speedup 1.0000x reference)
"""Trainium2 Bass kernel for nn_Net_34763465294339.

Four single-channel VALID convs (K=25/49/97/193, 16 output channels each) on
x[16,1,256,256], each squared + spatially averaged / scale -> stack -> fold
16 channels into 8 by adding halves. Output [16,8,4] f32.

Sharding: data-parallel over batch, 2 images per core, weights replicated.

Resident-window conv (v3): x rows stay in DRAM in dense layout; per
output-row block a [rows, planes, cols] window tile is DMA'd with large
contiguous per-partition runs (no im2col gather). Kernel-column shifts are
expressed as overlapping column offsets in the matmul rhs AP; kernel-row
shifts live in zero-padded stationary weights (contraction over window rows).

  K=25/49/97 run in fp8e4 with perf_mode=DoubleRow: contraction packs
  (g-replica, row) on partitions x 2 interleave planes, giving 2G kernel
  columns (dj) per matmul. Window planes are pre-shifted by one column so the
  dj pair comes from the plane dim. Per-block/dj0 weights are AP slices of
  one padded matrix per conv ([(g,r), dj0, i, (u,o)] with u = 8*t + s for
  multi-block windows). The 1/(S^2*scale) factor is applied as the
  activation pre-scale (fp8 weights cannot be pre-scaled: underflow).

  v3 structure (measured PE stream rate ~0.45-0.54 ns/output-row state-
  dependent, Ldweights fully hidden -> bound by total streamed psum rows,
  613.5k/core, with HWDGE DMA-issue pacing the second-order cost):
  - windows are FLAT (no duplicated pre-shifted planes): the DR plane pair
    is expressed as an overlapping +2B (+8B for the 8-img group layout)
    stride in the matmul rhs AP, halving window DMA bytes/descriptors; one
    3D-AP DMA covers all G replicas of a chunk (per plane for conv193).
  - conv25 repacked NBW=1 (G=4, Rw=32, CH=8, ndj0=4): 4 MMs/block, 53.8k
    rows vs 69.6k. conv49 stays NBW=2 single-chunk (13 MMs/block): the
    denser 4-chunk 12 MM/block packing loses more to HWDGE issue time
    than it saves in PE rows. conv97 uses 6 row-chunks {18x5,14} with
    G=7/9: 41 MMs/block vs 42.
  - conv25/49 run window-outer (window's matmuls then its Squares) so
    psum-bank rotation pipelines acts under the next window's matmuls.
  - matmul loops keep ONE psum accumulation region active at a time
    (conv49 t-outer, conv97 jc-outer): alternating psum targets
    mid-accumulation measurably costs ~10% of PE stream rate.
  - conv193 runs block-outer so blk0's Squares hide under blk1's matmuls.
  - conv49/conv25 windows emit interleaved (13:25) so the two independent
    DMA queues (sync HWDGE / Pool SWDGE) face evenly-paced demand; the
    last 5 conv25 windows + last conv49 window are held back as PE
    filler at the conv97 block boundaries and the 97->193 transition,
    absorbing boundary stalls.
  - fold/reduce/output are split per conv phase and interleaved into later
    convs' matmul streams (25/49+97 folds between the conv193 blocks);
    output DMAs ride the Act HWDGE queue so the sync window queue is
    never head-blocked across iterations.

  K=193 also runs fp8-DR with the pair planes carrying row-halves
  (contraction pairs (p, p+100) cover all 200 window rows -> ONE matmul per
  dj; post-fold fp8 error 1.72% vs the 2e-2 gate, deterministic inputs).
  It is also resharded: quads of cores share an 8-image group, each core
  runs the SAME program blocks {0,1} on x8g whose content is row-shifted
  by 16*(core%4) at upload, so N = 8 img * 64 = 512 and each core emits
  per-image partial energies (out193) that the host sums across the quad.
  conv97 is likewise resharded (x8g97, 5 program blocks, j-chunks).
"""
import numpy as np
import ml_dtypes

import concourse.bass as bass
import concourse.bacc as bacc
import concourse.mybir as mybir
from concourse.tile import TileContext
from concourse.bass_utils import run_bass_kernel_spmd

BF16 = mybir.dt.bfloat16
FP8 = mybir.dt.float8e4
F32 = mybir.dt.float32
NP_FP8 = ml_dtypes.float8_e4m3
NP_BF16 = ml_dtypes.bfloat16

IMG = 256
X8ROWS = 292  # padded rows for window reads past image end
NCORES = 8
BLOCK_I = 8

# fp8 convs: K -> (NBW, scale, chunks); each chunk is a row-range of the
# contraction packed as (G, Rw, CH, ndj0, off): window rows off..off+Rw-1,
# G column-replicas, dj = CH*g + 2*dj0 + i. conv97 splits its 104-row span
# into two chunks so dj-packing rises from 2/MM to 4-6/MM.
FP8_CONVS = {
    25: dict(NBW=1, scale=1.0,
             chunks=[dict(G=4, Rw=32, CH=8, ndj0=4, off=0)]),
    49: dict(NBW=2, scale=2.0,
             chunks=[dict(G=2, Rw=64, CH=26, ndj0=13, off=0)]),
    97: dict(NBW=1, scale=4.0,
             chunks=[dict(G=7, Rw=18, CH=14, ndj0=7, off=o)
                     for o in (0, 18, 36, 54, 72)]
             + [dict(G=9, Rw=14, CH=12, ndj0=6, off=90)]),
}
# processing order: conv49 first so conv25's window DMAs (own queue) run
# ahead during conv49's PE time
FP8_ORDER = (49, 25)
WIN_BUFS = {25: 6, 49: 8}
K193_SCALE = 8.0
CONVS = [25, 49, 97, 193]


def _S(K):
    return IMG - K + 1


def _F(K, ch):
    """fp8 flat window free size: max rhs read 4*(ndj0-1) + 2S + 2 (the DR
    plane pair is an overlapping +2B shift expressed in the rhs AP)."""
    f = 4 * (ch['ndj0'] - 1) + 2 * _S(K) + 2
    return (f + 15) // 16 * 16


def build_fp8_w(w, K, ch, NBW):
    """w: [16,K,K] f32 raw. Returns [G*Rw, ndj0*2*U*16] fp8 where
    M[(g,r), dj0, i, (u,o)] = w[o, off+r-u, CH*g+2*dj0+i], U = 8*NBW."""
    G, Rw, CH, ndj0, off = ch['G'], ch['Rw'], ch['CH'], ch['ndj0'], ch['off']
    U = 8 * NBW
    M = np.zeros((G, Rw, ndj0, 2, U, 16), np.float32)
    r = np.arange(Rw)
    for g in range(G):
        for dj0 in range(ndj0):
            for i in range(2):
                dj = CH * g + 2 * dj0 + i
                if dj >= K:
                    continue
                for u in range(U):
                    d = off + r - u
                    valid = (d >= 0) & (d < K)
                    M[g, r[valid], dj0, i, u, :] = w[:, d[valid], dj].T
    return M.reshape(G * Rw, ndj0 * 2 * U * 16).astype(NP_FP8)


# conv193 contraction chunks: rows off..off+2*Rw-1 as pairs (r, r+Rw),
# G dj-replicas, dj = CH*g + dj0
W193_CHUNKS = [dict(G=7, Rw=18, CH=28, off=0),
               dict(G=7, Rw=18, CH=28, off=36),
               dict(G=2, Rw=64, CH=97, off=72)]


def build_w193(w, ch):
    """w: [16,193,193] f32 RAW. Returns [G*Rw, CH*2*128] fp8 with
    M[(g,r), dj0, i, (s,o)] = w[o, off + r + Rw*i - s, CH*g + dj0]."""
    G, Rw, CH, off = ch['G'], ch['Rw'], ch['CH'], ch['off']
    M = np.zeros((G, Rw, CH, 2, 8, 16), np.float32)
    r = np.arange(Rw)
    for g in range(G):
        for s in range(8):
            for i in range(2):
                d = off + r + Rw * i - s
                v = (d >= 0) & (d < 193)
                for dj0 in range(CH):
                    dj = CH * g + dj0
                    if dj >= 193:
                        continue
                    M[g, r[v], dj0, i, s, :] = w[:, d[v], dj].T
    return M.reshape(G * Rw, CH * 2 * 128).astype(NP_FP8)


def _build_fold():
    F = np.zeros((128, 8), dtype=np.float32)
    for p in range(128):
        F[p, (p % 16) % 8] = 1.0
    return F


def _col_layout():
    """fp8 convs: (K, b) -> base col, width nb. conv97: (97,) -> base col,
    then col = base + img*15 + blk*3 + jc. conv193: (193,) -> base col,
    col = base + img*2 + blk."""
    col_base = {}
    c = 0
    for K in (25, 49):
        nb = _S(K) // BLOCK_I
        for b in range(2):
            col_base[(K, b)] = c
            c += nb
    c2549 = c
    col_base[(97,)] = c
    c += 8 * 15  # img*15 + blk*3 + jc
    c97 = c
    col_base[(193,)] = c
    c += 16
    return col_base, c, c2549, c97


def build_in_maps(x, w0, w1, w2, w3):
    """Full inputs -> per-core input dicts for the compiled nc."""
    x = np.asarray(x, dtype=np.float32).reshape(16, IMG, IMG)
    ws = {25: w0, 49: w1, 97: w2, 193: w3}

    shared = {}
    for K in (25, 49, 97):
        w = np.asarray(ws[K], dtype=np.float32).reshape(16, K, K)
        for ci_, ch in enumerate(FP8_CONVS[K]['chunks']):
            shared[f"w{K}_{ci_}"] = build_fp8_w(w, K, ch,
                                                FP8_CONVS[K]['NBW'])
    w = np.asarray(ws[193], dtype=np.float32).reshape(16, 193, 193)
    for ci_, ch in enumerate(W193_CHUNKS):
        shared[f"w193_{ci_}"] = build_w193(w, ch)
    shared["fold"] = _build_fold()

    in_maps = []
    for c in range(NCORES):
        m = dict(shared)
        # [row, (col, b)] interleaved pair of images
        pair = np.ascontiguousarray(
            x[2 * c:2 * c + 2].transpose(1, 2, 0)).reshape(IMG, 2 * IMG)
        x8 = np.zeros((X8ROWS, 2 * IMG), np.float32)
        x8[:IMG] = pair
        m["x8"] = x8.astype(NP_FP8)
        # conv193 group input: 8 images of group c//4, row-shifted by
        # 16*(c%4) so program blocks {0,1} compute real blocks 2*(c%4)+{0,1}
        g = c // 4
        oct_ = np.zeros((IMG + 3, 8 * IMG), np.float32)
        oct_[:IMG] = np.ascontiguousarray(
            x[8 * g:8 * g + 8].transpose(1, 2, 0)).reshape(IMG, 8 * IMG)
        r0 = 16 * (c % 4)
        m["x8g"] = np.ascontiguousarray(
            oct_[r0:r0 + 209]).astype(NP_FP8)
        # conv97 group input: same 8 images, row shift 40*(c%4) so program
        # blocks {0..4} compute real blocks 5*(c%4)+{0..4}
        r0 = 40 * (c % 4)
        m["x8g97"] = np.ascontiguousarray(
            oct_[r0:r0 + 138]).astype(NP_FP8)
        in_maps.append(m)
    return in_maps


QSPLIT49 = True


def _build_nc(repeat=1, only=None):
    nc = bacc.Bacc("TRN2", target_bir_lowering=False)
    x8 = nc.dram_tensor("x8", [X8ROWS, 2 * IMG], FP8, kind="ExternalInput")
    x8g = nc.dram_tensor("x8g", [209, 8 * IMG], FP8, kind="ExternalInput")
    x8g97 = nc.dram_tensor("x8g97", [138, 8 * IMG], FP8,
                           kind="ExternalInput")
    w_h = {}
    for K in (25, 49, 97):
        c = FP8_CONVS[K]
        for ci_, ch in enumerate(c['chunks']):
            w_h[(K, ci_)] = nc.dram_tensor(
                f"w{K}_{ci_}",
                [ch['G'] * ch['Rw'], ch['ndj0'] * 2 * 8 * c['NBW'] * 16],
                FP8, kind="ExternalInput")
    w193_h = {}
    for ci_, ch in enumerate(W193_CHUNKS):
        w193_h[ci_] = nc.dram_tensor(
            f"w193_{ci_}", [ch['G'] * ch['Rw'], ch['CH'] * 2 * 128], FP8,
            kind="ExternalInput")
    fold_h = nc.dram_tensor("fold", [128, 8], F32, kind="ExternalInput")
    out = nc.dram_tensor("out", [2, 8, 4], F32, kind="ExternalOutput")
    out193 = nc.dram_tensor("out193", [8, 8], F32, kind="ExternalOutput")
    out97 = nc.dram_tensor("out97", [8, 8], F32, kind="ExternalOutput")

    col_base, TOT, C2549, C97 = _col_layout()
    SQ = mybir.ActivationFunctionType.Square
    DR = mybir.MatmulPerfMode.DoubleRow

    with TileContext(nc) as tc:
        with tc.tile_pool(name="consts", bufs=1) as cpool, \
             tc.tile_pool(name="winp", bufs=2) as rpool, \
             tc.tile_pool(name="scrp", bufs=4) as spool, \
             tc.tile_pool(name="accp", bufs=8, space="PSUM") as ppool:
            w_sb = {}
            for key, h in w_h.items():
                K, ci_ = key
                t = cpool.tile(list(h.shape), FP8, name=f"w{K}_{ci_}sb",
                               tag=f"w{K}_{ci_}")
                nc.gpsimd.dma_start(out=t[:], in_=h[:])
                w_sb[key] = t
            w193 = {}
            for ci_, ch in enumerate(W193_CHUNKS):
                t = cpool.tile(list(w193_h[ci_].shape), FP8,
                               name=f"w193_{ci_}sb", tag=f"w193_{ci_}")
                nc.gpsimd.dma_start(out=t[:], in_=w193_h[ci_][:])
                w193[ci_] = t
            fold_sb = cpool.tile([128, 8], F32, name="fold_sb", tag="fold")
            nc.sync.dma_start(out=fold_sb[:], in_=fold_h[:])
            stage = cpool.tile([128, TOT], F32, name="stage", tag="stage")

            rep = tc.For_i(0, repeat) if repeat != 1 else None
            if rep is not None:
                rep.__enter__()

            # --- deferred post-processing emitters -------------------------
            res = spool.tile([8, 8], F32, name="res", tag="res", bufs=1)
            res97 = spool.tile([8, 8], F32, name="res97", tag="res97",
                               bufs=1)
            res193 = spool.tile([8, 8], F32, name="res193", tag="res193",
                                bufs=1)

            def post_2549():
                # fold (s,o) partitions -> o%8 over the 25/49 stage range,
                # reduce per (conv, image) col group, DMA out on Act HWDGE
                fps = ppool.tile([8, C2549], F32, name="fold_ps1", tag="acc")
                nc.tensor.matmul(fps[:], fold_sb[:], stage[:, 0:C2549],
                                 start=True, stop=True)
                for ci, K in enumerate((25, 49)):
                    nb = _S(K) // BLOCK_I
                    for b in range(2):
                        c0 = col_base[(K, b)]
                        oc = b * 4 + ci
                        nc.vector.reduce_sum(out=res[:8, oc:oc + 1],
                                             in_=fps[:8, c0:c0 + nb],
                                             axis=mybir.AxisListType.X)
                a = res[:8, :]
                src = bass.AP(a.tensor, a.offset,
                              [list(a.ap[0]), [4, 2], [1, 2]])
                dst = bass.AP(out, 0, [[4, 8], [32, 2], [1, 2]])
                nc.scalar.dma_start(out=dst, in_=src)

            def post_97():
                fps = ppool.tile([8, 120], F32, name="fold_ps97", tag="acc")
                nc.tensor.matmul(fps[:], fold_sb[:],
                                 stage[:, col_base[(97,)]:C97],
                                 start=True, stop=True)
                for i in range(8):
                    nc.vector.reduce_sum(
                        out=res97[:8, i:i + 1],
                        in_=fps[:8, 15 * i:15 * i + 15],
                        axis=mybir.AxisListType.X)
                dst97 = bass.AP(out97, 0, [[1, 8], [8, 8]])
                nc.scalar.dma_start(out=dst97, in_=res97[:8, :])

            def post_193():
                fps = ppool.tile([8, 16], F32, name="fold_ps193", tag="acc")
                nc.tensor.matmul(fps[:], fold_sb[:],
                                 stage[:, col_base[(193,)]:TOT],
                                 start=True, stop=True)
                for i in range(8):
                    nc.vector.reduce_sum(
                        out=res193[:8, i:i + 1],
                        in_=fps[:8, 2 * i:2 * i + 2],
                        axis=mybir.AxisListType.X)
                dst193 = bass.AP(out193, 0, [[1, 8], [8, 8]])
                nc.scalar.dma_start(out=dst193, in_=res193[:8, :])

            # --- fp8 DoubleRow convs 49 + 25, window-outer, interleaved ----
            def _fp8_ctx(K):
                c = FP8_CONVS[K]
                chunks = c['chunks']
                S = _S(K)
                nb = S // BLOCK_I
                return dict(
                    K=K, NBW=c['NBW'], chunks=chunks, S=S, nb=nb,
                    act_scale=float(
                        np.sqrt(1.0 / (float(S) ** 2 * c['scale']))),
                    nwin=(nb + c['NBW'] - 1) // c['NBW'],
                    dma_eng=nc.gpsimd if K == 25 else nc.sync,
                    wms=[w_sb[(K, ci_)].rearrange(
                        "p (d i m) -> p d i m", d=ch['ndj0'], i=2)
                        for ci_, ch in enumerate(chunks)])

            def _fp8_win(ctx, wi):
                K, NBW, chunks, S = (ctx['K'], ctx['NBW'], ctx['chunks'],
                                     ctx['S'])
                nchunks = len(chunks)
                i0 = wi * NBW * BLOCK_I
                nts = min(NBW, ctx['nb'] - wi * NBW)
                wins = {}
                for ci_, ch in enumerate(chunks):
                    G, Rw, CH = ch['G'], ch['Rw'], ch['CH']
                    F = _F(K, ch)
                    win = rpool.tile([G * Rw, F], FP8,
                                     name=f"win{K}_{ci_}_{wi}",
                                     tag=f"win{K}_{ci_}",
                                     bufs=WIN_BUFS[K])
                    # one flat 3D-AP DMA covering all G replicas; the
                    # DR plane pair is read later as a +2B overlap
                    src = bass.AP(
                        x8, (i0 + ch['off']) * 2 * IMG,
                        [[2 * CH, G], [2 * IMG, Rw], [1, F]])
                    ctx['dma_eng'].dma_start(out=win[:], in_=src)
                    wins[ci_] = win[:]
                psums = {}
                for t in range(nts):
                    psums[t] = ppool.tile([128, 2 * S], F32,
                                          name=f"ps{K}_{wi}_{t}",
                                          tag="acc")
                for t in range(nts):
                    for ci_, ch in enumerate(chunks):
                        ndj0 = ch['ndj0']
                        a = wins[ci_]
                        for dj0 in range(ndj0):
                            rhs = bass.AP(
                                a.tensor, a.offset + 4 * dj0,
                                [list(a.ap[0]), [2, 2], [1, 2 * S]])
                            lhsT = ctx['wms'][ci_][
                                :, dj0, :, 8 * t * 16:8 * t * 16 + 128]
                            nc.tensor.matmul(
                                psums[t][:], lhsT, rhs,
                                start=(ci_ == 0 and dj0 == 0),
                                stop=(ci_ == nchunks - 1
                                      and dj0 == ndj0 - 1),
                                perf_mode=DR)
                for t in range(nts):
                    blk = wi * NBW + t
                    for b in range(2):
                        scr = spool.tile([128, S], F32,
                                         name=f"sq{K}_{blk}_{b}",
                                         tag="scr")
                        col = col_base[(K, b)] + blk
                        nc.scalar.activation(
                            out=scr[:], in_=psums[t][:, b::2],
                            func=SQ, scale=ctx['act_scale'],
                            accum_out=stage[:, col:col + 1])

            ks = [k for k in FP8_ORDER if only in (None, k)]
            defer25 = []
            if len(ks) == 2:
                cA, cB = _fp8_ctx(49), _fp8_ctx(25)
                # hold 5 conv25 windows + the last conv49 window back as PE
                # filler at the conv97 block boundaries and the 97->193
                # transition
                nB = cB['nwin'] - 5
                nA = cA['nwin'] - 1
                done_b = 0
                for wiA in range(nA):
                    _fp8_win(cA, wiA)
                    want = (wiA + 1) * nB // nA
                    while done_b < want:
                        _fp8_win(cB, done_b)
                        done_b += 1
                defer25 = ([(cB, wi) for wi in range(nB, cB['nwin'])]
                           + [(cA, nA)])
            else:
                for K in ks:
                    ctx = _fp8_ctx(K)
                    for wi in range(ctx['nwin']):
                        _fp8_win(ctx, wi)

            # --- fp8-DR conv K=97, resharded: 5 blocks x 8 group images ----
            S97 = _S(97)
            act97 = float(np.sqrt(1.0 / (float(S97) ** 2 * 4.0)))
            ch97 = FP8_CONVS[97]['chunks']
            wms97 = [w_sb[(97, ci_)].rearrange(
                "p (d i m) -> p d i m", d=ch['ndj0'], i=2)
                for ci_, ch in enumerate(ch97)]
            JC97 = [(0, 64), (64, 64), (128, 32)]
            n97 = len(ch97)
            F97 = 8 * IMG + 8
            for blk in (range(5) if only in (None, 97) else ()):
                i0 = blk * BLOCK_I
                wins97 = {}
                for ci_, ch in enumerate(ch97):
                    G, Rw, CH, off = ch['G'], ch['Rw'], ch['CH'], ch['off']
                    win = rpool.tile([G * Rw, F97], FP8,
                                     name=f"win97g_{ci_}_{blk}",
                                     tag=f"win97g_{ci_}", bufs=4)
                    # flat window; DR plane pair read as +8B overlap
                    src = bass.AP(
                        x8g97, (i0 + off) * 8 * IMG,
                        [[8 * CH, G], [8 * IMG, Rw], [1, F97]])
                    nc.sync.dma_start(out=win[:], in_=src)
                    wins97[ci_] = win[:]
                ps97 = {}
                for jc, (j0, jl) in enumerate(JC97):
                    ps97[jc] = ppool.tile([128, 8 * jl], F32,
                                          name=f"ps97_{blk}_{jc}", tag="acc")
                for jc, (j0, jl) in enumerate(JC97):
                    for ci_, ch in enumerate(ch97):
                        ndj0 = ch['ndj0']
                        a = wins97[ci_]
                        for dj0 in range(ndj0):
                            lhsT = wms97[ci_][:, dj0, :, 0:128]
                            rhs = bass.AP(
                                a.tensor,
                                a.offset + 16 * dj0 + 8 * j0,
                                [list(a.ap[0]), [8, 2], [1, 8 * jl]])
                            nc.tensor.matmul(
                                ps97[jc][:], lhsT, rhs,
                                start=(ci_ == 0 and dj0 == 0),
                                stop=(ci_ == n97 - 1 and dj0 == ndj0 - 1),
                                perf_mode=DR)
                if blk >= 1 and defer25:
                    _fp8_win(*defer25.pop(0))
                for jc, (j0, jl) in enumerate(JC97):
                    for b in range(8):
                        scr = spool.tile([128, jl], F32,
                                         name=f"sq97_{blk}_{jc}_{b}",
                                         tag="scr")
                        col = col_base[(97,)] + b * 15 + blk * 3 + jc
                        nc.scalar.activation(
                            out=scr[:], in_=ps97[jc][:, b::8], func=SQ,
                            scale=act97,
                            accum_out=stage[:, col:col + 1])

            # --- fp8-DR conv K=193: 2 blocks x 8 group images, blk-outer ---
            S = _S(193)
            act193 = float(np.sqrt(1.0 / (float(S) ** 2 * K193_SCALE)))
            w193r = [w193[ci_].rearrange("p (d i m) -> p d i m",
                                         d=ch['CH'], i=2)
                     for ci_, ch in enumerate(W193_CHUNKS)]
            for blk in (range(2) if only in (None, 193) else ()):
                i0 = blk * BLOCK_I
                wins193 = {}
                for ci_, ch in enumerate(W193_CHUNKS):
                    G, Rw, CH, off = ch['G'], ch['Rw'], ch['CH'], ch['off']
                    F = 8 * (CH - 1) + 8 * S
                    win = rpool.tile([G * Rw, 2 * F], FP8,
                                     name=f"win193_{ci_}_{blk}",
                                     tag=f"win193_{ci_}", bufs=3)
                    for i_ in range(2):
                        src = bass.AP(
                            x8g,
                            (i0 + off + Rw * i_) * 8 * IMG,
                            [[8 * CH, G], [8 * IMG, Rw], [1, F]])
                        nc.sync.dma_start(
                            out=win[:, i_ * F:(i_ + 1) * F], in_=src)
                    wins193[ci_] = win.rearrange("p (i f) -> p i f", i=2)
                ps193 = ppool.tile([128, 8 * S], F32,
                                   name=f"ps193_{blk}", tag="acc")
                for ci_, ch in enumerate(W193_CHUNKS):
                    CH = ch['CH']
                    for dj0 in range(CH):
                        nc.tensor.matmul(
                            ps193[:], w193r[ci_][:, dj0, :, :],
                            wins193[ci_][:, :, 8 * dj0:8 * dj0 + 8 * S],
                            start=(ci_ == 0 and dj0 == 0),
                            stop=(ci_ == len(W193_CHUNKS) - 1
                                  and dj0 == CH - 1),
                            perf_mode=DR)
                if blk == 0 and only is None:
                    while defer25:
                        _fp8_win(*defer25.pop(0))
                    # 25/49 + 97 stage cols complete: fold+reduce+out here
                    # so only conv193's own post-processing tails the iter
                    post_2549()
                    post_97()
                for b in range(8):
                    scr = spool.tile([128, S], F32, name=f"sq193_{blk}_{b}",
                                     tag="scr")
                    col = col_base[(193,)] + b * 2 + blk
                    nc.scalar.activation(
                        out=scr[:], in_=ps193[:, b::8], func=SQ,
                        scale=act193,
                        accum_out=stage[:, col:col + 1])
            if only is None:
                post_193()
            if rep is not None:
                rep.__exit__(None, None, None)
    return nc


def _coalesce_pe_sem_incs(nc):
    """Drop per-matmul PE semaphore increments except on stop_tensor_calc
    matmuls, remapping every wait to the kept-increment count.

    Matmuls complete in program order, and (asserted below) every wait value
    on a matmul-produced semaphore lands exactly on a stop matmul, so the
    remapped wait fires at the completion of the same instruction. This
    removes ~1650 serialized EVT_SEM writes (~26ns each) from the PE stream.
    """
    f = nc.m.functions[0]
    # collect per-semaphore update lists (program order within each block)
    upd = {}
    blk_of = {}
    nonmm = set()
    waits_on = {}
    for bi, bb in enumerate(f.blocks):
        for inst in bb.instructions:
            si = inst.sync_info
            if si is None:
                continue
            for u in (si.on_update or []):
                if u.sync_type != 'semaphore':
                    continue
                if type(inst).__name__ == 'InstMatmult':
                    upd.setdefault(u.id, []).append(
                        (inst, bool(inst.stop_tensor_calc), u))
                    if u.id in blk_of and blk_of[u.id] != bi:
                        nonmm.add(u.id)
                    blk_of[u.id] = bi
                else:
                    nonmm.add(u.id)
            for w in (si.on_wait or []):
                if w.sync_type == 'semaphore':
                    waits_on.setdefault(w.id, []).append((inst, w))
    stripped = remapped = 0
    for sem_id, ups in upd.items():
        if sem_id in nonmm:
            continue
        ws = waits_on.get(sem_id, [])
        ok = all(u.update_mode == 'sem-inc' and u.update_value == 1
                 and u.update_reg is None for _, _, u in ups)
        pref = [0]
        for _, st, _ in ups:
            pref.append(pref[-1] + (1 if st else 0))
        nstops = pref[-1]

        def _new_wait(V):
            # stops-count to wait for: the stop at-or-after update V
            # (waiting on a later instruction is always safe)
            return pref[V] if ups[V - 1][1] else pref[V] + 1

        for _, w in ws:
            V = w.wait_value
            if (w.wait_mode != 'sem-ge-imm' or w.wait_reg is not None
                    or V is None or V < 1 or V > len(ups)
                    or _new_wait(V) > nstops):
                ok = False
        if not ok:
            continue
        for winst, w in ws:
            si = winst.sync_info
            new_waits = []
            for ow in si.on_wait:
                if ow.sync_type == 'semaphore' and ow.id == sem_id:
                    new_waits.append(mybir.SyncWait(
                        sync_type=ow.sync_type, id=ow.id,
                        ant_name=ow.ant_name, wait_mode=ow.wait_mode,
                        wait_value=_new_wait(ow.wait_value),
                        wait_reg=ow.wait_reg))
                    remapped += 1
                else:
                    new_waits.append(ow)
            winst.sync_info = mybir.SyncInfo(
                on_wait=new_waits, on_update=list(si.on_update or []))
        for inst, st, u in ups:
            if st:
                continue
            si = inst.sync_info
            new_upd = [ou for ou in si.on_update
                       if not (ou.sync_type == 'semaphore'
                               and ou.id == sem_id)]
            inst.sync_info = mybir.SyncInfo(
                on_wait=list(si.on_wait or []), on_update=new_upd)
            stripped += 1
    return stripped, remapped


_NC_CACHE = {}


def _get_nc(repeat=1):
    if repeat not in _NC_CACHE:
        nc = _build_nc(repeat=repeat)
        _coalesce_pe_sem_incs(nc)
        nc.compile()
        _NC_CACHE[repeat] = nc
    return _NC_CACHE[repeat]


def kernel(x, w0, w1, w2, w3):
    in_maps = build_in_maps(x, w0, w1, w2, w3)
    nc = _get_nc()
    r = run_bass_kernel_spmd(nc, in_maps, list(range(NCORES)))
    final = np.concatenate([np.asarray(r.results[c]["out"], dtype=np.float32)
                            for c in range(NCORES)], axis=0)
    for g in range(2):
        p = sum(np.asarray(r.results[4 * g + j]["out193"], dtype=np.float32)
                for j in range(4))
        final[8 * g:8 * g + 8, :, 3] = p
        p = sum(np.asarray(r.results[4 * g + j]["out97"], dtype=np.float32)
                for j in range(4))
        final[8 * g:8 * g + 8, :, 2] = p
    return final


# revision 29
# speedup vs baseline: 1.0100x; 1.0100x over previous
"""Trainium2 Bass kernel for nn_Net_34763465294339.

Four single-channel VALID convs (K=25/49/97/193, 16 output channels each) on
x[16,1,256,256], each squared + spatially averaged / scale -> stack -> fold
16 channels into 8 by adding halves. Output [16,8,4] f32.

Sharding: data-parallel over batch, 2 images per core, weights replicated.

Resident-window conv (v3): x rows stay in DRAM in dense layout; per
output-row block a [rows, planes, cols] window tile is DMA'd with large
contiguous per-partition runs (no im2col gather). Kernel-column shifts are
expressed as overlapping column offsets in the matmul rhs AP; kernel-row
shifts live in zero-padded stationary weights (contraction over window rows).

  K=25/49/97 run in fp8e4 with perf_mode=DoubleRow: contraction packs
  (g-replica, row) on partitions x 2 interleave planes, giving 2G kernel
  columns (dj) per matmul. Window planes are pre-shifted by one column so the
  dj pair comes from the plane dim. Per-block/dj0 weights are AP slices of
  one padded matrix per conv ([(g,r), dj0, i, (u,o)] with u = 8*t + s for
  multi-block windows). The 1/(S^2*scale) factor is applied as the
  activation pre-scale (fp8 weights cannot be pre-scaled: underflow).

  v3 structure (measured PE stream rate ~0.45-0.54 ns/output-row state-
  dependent, Ldweights fully hidden -> bound by total streamed psum rows,
  613.5k/core, with HWDGE DMA-issue pacing the second-order cost):
  - windows are FLAT (no duplicated pre-shifted planes): the DR plane pair
    is expressed as an overlapping +2B (+8B for the 8-img group layout)
    stride in the matmul rhs AP, halving window DMA bytes/descriptors; one
    3D-AP DMA covers all G replicas of a chunk (per plane for conv193).
  - conv25 repacked NBW=1 (G=4, Rw=32, CH=8, ndj0=4): 4 MMs/block, 53.8k
    rows vs 69.6k. conv49 stays NBW=2 single-chunk (13 MMs/block): the
    denser 4-chunk 12 MM/block packing loses more to HWDGE issue time
    than it saves in PE rows. conv97 uses 6 row-chunks {18x5,14} with
    G=7/9: 41 MMs/block vs 42.
  - conv25/49 run window-outer (window's matmuls then its Squares) so
    psum-bank rotation pipelines acts under the next window's matmuls.
  - matmul loops keep ONE psum accumulation region active at a time
    (conv49 t-outer, conv97 jc-outer): alternating psum targets
    mid-accumulation measurably costs ~10% of PE stream rate.
  - conv193 runs block-outer so blk0's Squares hide under blk1's matmuls.
  - conv49/conv25 windows emit interleaved (13:25) so the two independent
    DMA queues (sync HWDGE / Pool SWDGE) face evenly-paced demand; the
    last 5 conv25 windows + last conv49 window are held back as PE
    filler at the conv97 block boundaries and the conv193 blk0->blk1
    boundary (measured: the 97->193 transition itself does not stall).
  - fold/reduce/output are split per conv phase and interleaved into later
    convs' matmul streams (25/49+97 folds between the conv193 blocks);
    output DMAs ride the Act HWDGE queue so the sync window queue is
    never head-blocked across iterations.

  K=193 also runs fp8-DR with the pair planes carrying row-halves
  (contraction pairs (p, p+100) cover all 200 window rows -> ONE matmul per
  dj; post-fold fp8 error 1.72% vs the 2e-2 gate, deterministic inputs).
  It is also resharded: quads of cores share an 8-image group, each core
  runs the SAME program blocks {0,1} on x8g whose content is row-shifted
  by 16*(core%4) at upload, so N = 8 img * 64 = 512 and each core emits
  per-image partial energies (out193) that the host sums across the quad.
  conv97 is likewise resharded (x8g97, 5 program blocks, j-chunks).
"""
import numpy as np
import ml_dtypes

import concourse.bass as bass
import concourse.bacc as bacc
import concourse.mybir as mybir
from concourse.tile import TileContext
from concourse.bass_utils import run_bass_kernel_spmd

BF16 = mybir.dt.bfloat16
FP8 = mybir.dt.float8e4
F32 = mybir.dt.float32
NP_FP8 = ml_dtypes.float8_e4m3
NP_BF16 = ml_dtypes.bfloat16

IMG = 256
X8ROWS = 292  # padded rows for window reads past image end
NCORES = 8
BLOCK_I = 8

# fp8 convs: K -> (NBW, scale, chunks); each chunk is a row-range of the
# contraction packed as (G, Rw, CH, ndj0, off): window rows off..off+Rw-1,
# G column-replicas, dj = CH*g + 2*dj0 + i. conv97 splits its 104-row span
# into two chunks so dj-packing rises from 2/MM to 4-6/MM.
FP8_CONVS = {
    25: dict(NBW=1, scale=1.0,
             chunks=[dict(G=4, Rw=32, CH=8, ndj0=4, off=0)]),
    49: dict(NBW=2, scale=2.0,
             chunks=[dict(G=2, Rw=64, CH=26, ndj0=13, off=0)]),
    97: dict(NBW=1, scale=4.0,
             chunks=[dict(G=7, Rw=18, CH=14, ndj0=7, off=o)
                     for o in (0, 18, 36, 54, 72)]
             + [dict(G=9, Rw=14, CH=12, ndj0=6, off=90)]),
}
# processing order: conv49 first so conv25's window DMAs (own queue) run
# ahead during conv49's PE time
FP8_ORDER = (49, 25)
WIN_BUFS = {25: 6, 49: 8}
K193_SCALE = 8.0
CONVS = [25, 49, 97, 193]


def _S(K):
    return IMG - K + 1


def _F(K, ch):
    """fp8 flat window free size: max rhs read 4*(ndj0-1) + 2S + 2 (the DR
    plane pair is an overlapping +2B shift expressed in the rhs AP)."""
    f = 4 * (ch['ndj0'] - 1) + 2 * _S(K) + 2
    return (f + 15) // 16 * 16


def build_fp8_w(w, K, ch, NBW):
    """w: [16,K,K] f32 raw. Returns [G*Rw, ndj0*2*U*16] fp8 where
    M[(g,r), dj0, i, (u,o)] = w[o, off+r-u, CH*g+2*dj0+i], U = 8*NBW."""
    G, Rw, CH, ndj0, off = ch['G'], ch['Rw'], ch['CH'], ch['ndj0'], ch['off']
    U = 8 * NBW
    M = np.zeros((G, Rw, ndj0, 2, U, 16), np.float32)
    r = np.arange(Rw)
    for g in range(G):
        for dj0 in range(ndj0):
            for i in range(2):
                dj = CH * g + 2 * dj0 + i
                if dj >= K:
                    continue
                for u in range(U):
                    d = off + r - u
                    valid = (d >= 0) & (d < K)
                    M[g, r[valid], dj0, i, u, :] = w[:, d[valid], dj].T
    return M.reshape(G * Rw, ndj0 * 2 * U * 16).astype(NP_FP8)


# conv193 contraction chunks: rows off..off+2*Rw-1 as pairs (r, r+Rw),
# G dj-replicas, dj = CH*g + dj0
W193_CHUNKS = [dict(G=7, Rw=18, CH=28, off=0),
               dict(G=7, Rw=18, CH=28, off=36),
               dict(G=2, Rw=64, CH=97, off=72)]


def build_w193(w, ch):
    """w: [16,193,193] f32 RAW. Returns [G*Rw, CH*2*128] fp8 with
    M[(g,r), dj0, i, (s,o)] = w[o, off + r + Rw*i - s, CH*g + dj0]."""
    G, Rw, CH, off = ch['G'], ch['Rw'], ch['CH'], ch['off']
    M = np.zeros((G, Rw, CH, 2, 8, 16), np.float32)
    r = np.arange(Rw)
    for g in range(G):
        for s in range(8):
            for i in range(2):
                d = off + r + Rw * i - s
                v = (d >= 0) & (d < 193)
                for dj0 in range(CH):
                    dj = CH * g + dj0
                    if dj >= 193:
                        continue
                    M[g, r[v], dj0, i, s, :] = w[:, d[v], dj].T
    return M.reshape(G * Rw, CH * 2 * 128).astype(NP_FP8)


def _build_fold():
    F = np.zeros((128, 8), dtype=np.float32)
    for p in range(128):
        F[p, (p % 16) % 8] = 1.0
    return F


def _col_layout():
    """fp8 convs: (K, b) -> base col, width nb. conv97: (97,) -> base col,
    then col = base + img*15 + blk*3 + jc. conv193: (193,) -> base col,
    col = base + img*2 + blk."""
    col_base = {}
    c = 0
    for K in (25, 49):
        nb = _S(K) // BLOCK_I
        for b in range(2):
            col_base[(K, b)] = c
            c += nb
    c2549 = c
    col_base[(97,)] = c
    c += 8 * 15  # img*15 + blk*3 + jc
    c97 = c
    col_base[(193,)] = c
    c += 16
    return col_base, c, c2549, c97


def build_in_maps(x, w0, w1, w2, w3):
    """Full inputs -> per-core input dicts for the compiled nc."""
    x = np.asarray(x, dtype=np.float32).reshape(16, IMG, IMG)
    ws = {25: w0, 49: w1, 97: w2, 193: w3}

    shared = {}
    for K in (25, 49, 97):
        w = np.asarray(ws[K], dtype=np.float32).reshape(16, K, K)
        for ci_, ch in enumerate(FP8_CONVS[K]['chunks']):
            shared[f"w{K}_{ci_}"] = build_fp8_w(w, K, ch,
                                                FP8_CONVS[K]['NBW'])
    w = np.asarray(ws[193], dtype=np.float32).reshape(16, 193, 193)
    for ci_, ch in enumerate(W193_CHUNKS):
        shared[f"w193_{ci_}"] = build_w193(w, ch)
    shared["fold"] = _build_fold()

    in_maps = []
    for c in range(NCORES):
        m = dict(shared)
        # [row, (col, b)] interleaved pair of images
        pair = np.ascontiguousarray(
            x[2 * c:2 * c + 2].transpose(1, 2, 0)).reshape(IMG, 2 * IMG)
        x8 = np.zeros((X8ROWS, 2 * IMG), np.float32)
        x8[:IMG] = pair
        m["x8"] = x8.astype(NP_FP8)
        # conv193 group input: 8 images of group c//4, row-shifted by
        # 16*(c%4) so program blocks {0,1} compute real blocks 2*(c%4)+{0,1}
        g = c // 4
        oct_ = np.zeros((IMG + 3, 8 * IMG), np.float32)
        oct_[:IMG] = np.ascontiguousarray(
            x[8 * g:8 * g + 8].transpose(1, 2, 0)).reshape(IMG, 8 * IMG)
        r0 = 16 * (c % 4)
        m["x8g"] = np.ascontiguousarray(
            oct_[r0:r0 + 209]).astype(NP_FP8)
        # conv97 group input: same 8 images, row shift 40*(c%4) so program
        # blocks {0..4} compute real blocks 5*(c%4)+{0..4}
        r0 = 40 * (c % 4)
        m["x8g97"] = np.ascontiguousarray(
            oct_[r0:r0 + 138]).astype(NP_FP8)
        in_maps.append(m)
    return in_maps


QSPLIT49 = True


def _build_nc(repeat=1, only=None):
    nc = bacc.Bacc("TRN2", target_bir_lowering=False)
    x8 = nc.dram_tensor("x8", [X8ROWS, 2 * IMG], FP8, kind="ExternalInput")
    x8g = nc.dram_tensor("x8g", [209, 8 * IMG], FP8, kind="ExternalInput")
    x8g97 = nc.dram_tensor("x8g97", [138, 8 * IMG], FP8,
                           kind="ExternalInput")
    w_h = {}
    for K in (25, 49, 97):
        c = FP8_CONVS[K]
        for ci_, ch in enumerate(c['chunks']):
            w_h[(K, ci_)] = nc.dram_tensor(
                f"w{K}_{ci_}",
                [ch['G'] * ch['Rw'], ch['ndj0'] * 2 * 8 * c['NBW'] * 16],
                FP8, kind="ExternalInput")
    w193_h = {}
    for ci_, ch in enumerate(W193_CHUNKS):
        w193_h[ci_] = nc.dram_tensor(
            f"w193_{ci_}", [ch['G'] * ch['Rw'], ch['CH'] * 2 * 128], FP8,
            kind="ExternalInput")
    fold_h = nc.dram_tensor("fold", [128, 8], F32, kind="ExternalInput")
    out = nc.dram_tensor("out", [2, 8, 4], F32, kind="ExternalOutput")
    out193 = nc.dram_tensor("out193", [8, 8], F32, kind="ExternalOutput")
    out97 = nc.dram_tensor("out97", [8, 8], F32, kind="ExternalOutput")

    col_base, TOT, C2549, C97 = _col_layout()
    SQ = mybir.ActivationFunctionType.Square
    DR = mybir.MatmulPerfMode.DoubleRow

    with TileContext(nc) as tc:
        with tc.tile_pool(name="consts", bufs=1) as cpool, \
             tc.tile_pool(name="winp", bufs=2) as rpool, \
             tc.tile_pool(name="scrp", bufs=4) as spool, \
             tc.tile_pool(name="accp", bufs=8, space="PSUM") as ppool:
            w_sb = {}
            for key, h in w_h.items():
                K, ci_ = key
                t = cpool.tile(list(h.shape), FP8, name=f"w{K}_{ci_}sb",
                               tag=f"w{K}_{ci_}")
                nc.gpsimd.dma_start(out=t[:], in_=h[:])
                w_sb[key] = t
            w193 = {}
            for ci_, ch in enumerate(W193_CHUNKS):
                t = cpool.tile(list(w193_h[ci_].shape), FP8,
                               name=f"w193_{ci_}sb", tag=f"w193_{ci_}")
                nc.gpsimd.dma_start(out=t[:], in_=w193_h[ci_][:])
                w193[ci_] = t
            fold_sb = cpool.tile([128, 8], F32, name="fold_sb", tag="fold")
            nc.sync.dma_start(out=fold_sb[:], in_=fold_h[:])
            stage = cpool.tile([128, TOT], F32, name="stage", tag="stage")

            rep = tc.For_i(0, repeat) if repeat != 1 else None
            if rep is not None:
                rep.__enter__()

            # --- deferred post-processing emitters -------------------------
            res = spool.tile([8, 8], F32, name="res", tag="res", bufs=1)
            res97 = spool.tile([8, 8], F32, name="res97", tag="res97",
                               bufs=1)
            res193 = spool.tile([8, 8], F32, name="res193", tag="res193",
                                bufs=1)

            def post_2549():
                # fold (s,o) partitions -> o%8 over the 25/49 stage range,
                # reduce per (conv, image) col group, DMA out on Act HWDGE
                fps = ppool.tile([8, C2549], F32, name="fold_ps1", tag="acc")
                nc.tensor.matmul(fps[:], fold_sb[:], stage[:, 0:C2549],
                                 start=True, stop=True)
                for ci, K in enumerate((25, 49)):
                    nb = _S(K) // BLOCK_I
                    for b in range(2):
                        c0 = col_base[(K, b)]
                        oc = b * 4 + ci
                        nc.vector.reduce_sum(out=res[:8, oc:oc + 1],
                                             in_=fps[:8, c0:c0 + nb],
                                             axis=mybir.AxisListType.X)
                a = res[:8, :]
                src = bass.AP(a.tensor, a.offset,
                              [list(a.ap[0]), [4, 2], [1, 2]])
                dst = bass.AP(out, 0, [[4, 8], [32, 2], [1, 2]])
                nc.scalar.dma_start(out=dst, in_=src)

            def post_97():
                fps = ppool.tile([8, 120], F32, name="fold_ps97", tag="acc")
                nc.tensor.matmul(fps[:], fold_sb[:],
                                 stage[:, col_base[(97,)]:C97],
                                 start=True, stop=True)
                for i in range(8):
                    nc.vector.reduce_sum(
                        out=res97[:8, i:i + 1],
                        in_=fps[:8, 15 * i:15 * i + 15],
                        axis=mybir.AxisListType.X)
                dst97 = bass.AP(out97, 0, [[1, 8], [8, 8]])
                nc.scalar.dma_start(out=dst97, in_=res97[:8, :])

            def post_193():
                fps = ppool.tile([8, 16], F32, name="fold_ps193", tag="acc")
                nc.tensor.matmul(fps[:], fold_sb[:],
                                 stage[:, col_base[(193,)]:TOT],
                                 start=True, stop=True)
                for i in range(8):
                    nc.vector.reduce_sum(
                        out=res193[:8, i:i + 1],
                        in_=fps[:8, 2 * i:2 * i + 2],
                        axis=mybir.AxisListType.X)
                dst193 = bass.AP(out193, 0, [[1, 8], [8, 8]])
                nc.scalar.dma_start(out=dst193, in_=res193[:8, :])

            # --- fp8 DoubleRow convs 49 + 25, window-outer, interleaved ----
            def _fp8_ctx(K):
                c = FP8_CONVS[K]
                chunks = c['chunks']
                S = _S(K)
                nb = S // BLOCK_I
                return dict(
                    K=K, NBW=c['NBW'], chunks=chunks, S=S, nb=nb,
                    act_scale=float(
                        np.sqrt(1.0 / (float(S) ** 2 * c['scale']))),
                    nwin=(nb + c['NBW'] - 1) // c['NBW'],
                    dma_eng=nc.gpsimd if K == 25 else nc.sync,
                    wms=[w_sb[(K, ci_)].rearrange(
                        "p (d i m) -> p d i m", d=ch['ndj0'], i=2)
                        for ci_, ch in enumerate(chunks)])

            def _fp8_win(ctx, wi):
                K, NBW, chunks, S = (ctx['K'], ctx['NBW'], ctx['chunks'],
                                     ctx['S'])
                nchunks = len(chunks)
                i0 = wi * NBW * BLOCK_I
                nts = min(NBW, ctx['nb'] - wi * NBW)
                wins = {}
                for ci_, ch in enumerate(chunks):
                    G, Rw, CH = ch['G'], ch['Rw'], ch['CH']
                    F = _F(K, ch)
                    win = rpool.tile([G * Rw, F], FP8,
                                     name=f"win{K}_{ci_}_{wi}",
                                     tag=f"win{K}_{ci_}",
                                     bufs=WIN_BUFS[K])
                    # one flat 3D-AP DMA covering all G replicas; the
                    # DR plane pair is read later as a +2B overlap
                    src = bass.AP(
                        x8, (i0 + ch['off']) * 2 * IMG,
                        [[2 * CH, G], [2 * IMG, Rw], [1, F]])
                    ctx['dma_eng'].dma_start(out=win[:], in_=src)
                    wins[ci_] = win[:]
                psums = {}
                for t in range(nts):
                    psums[t] = ppool.tile([128, 2 * S], F32,
                                          name=f"ps{K}_{wi}_{t}",
                                          tag="acc")
                for t in range(nts):
                    for ci_, ch in enumerate(chunks):
                        ndj0 = ch['ndj0']
                        a = wins[ci_]
                        for dj0 in range(ndj0):
                            rhs = bass.AP(
                                a.tensor, a.offset + 4 * dj0,
                                [list(a.ap[0]), [2, 2], [1, 2 * S]])
                            lhsT = ctx['wms'][ci_][
                                :, dj0, :, 8 * t * 16:8 * t * 16 + 128]
                            nc.tensor.matmul(
                                psums[t][:], lhsT, rhs,
                                start=(ci_ == 0 and dj0 == 0),
                                stop=(ci_ == nchunks - 1
                                      and dj0 == ndj0 - 1),
                                perf_mode=DR)
                for t in range(nts):
                    blk = wi * NBW + t
                    for b in range(2):
                        scr = spool.tile([128, S], F32,
                                         name=f"sq{K}_{blk}_{b}",
                                         tag="scr")
                        col = col_base[(K, b)] + blk
                        nc.scalar.activation(
                            out=scr[:], in_=psums[t][:, b::2],
                            func=SQ, scale=ctx['act_scale'],
                            accum_out=stage[:, col:col + 1])

            ks = [k for k in FP8_ORDER if only in (None, k)]
            defer25 = []
            if len(ks) == 2:
                cA, cB = _fp8_ctx(49), _fp8_ctx(25)
                # hold 5 conv25 windows + the last conv49 window back as PE
                # filler at the conv97 block boundaries and the 97->193
                # transition
                nB = cB['nwin'] - 5
                nA = cA['nwin'] - 1
                done_b = 0
                for wiA in range(nA):
                    _fp8_win(cA, wiA)
                    want = (wiA + 1) * nB // nA
                    while done_b < want:
                        _fp8_win(cB, done_b)
                        done_b += 1
                defer25 = ([(cB, wi) for wi in range(nB, cB['nwin'])]
                           + [(cA, nA)])
            else:
                for K in ks:
                    ctx = _fp8_ctx(K)
                    for wi in range(ctx['nwin']):
                        _fp8_win(ctx, wi)

            # --- fp8-DR conv K=97, resharded: 5 blocks x 8 group images ----
            S97 = _S(97)
            act97 = float(np.sqrt(1.0 / (float(S97) ** 2 * 4.0)))
            ch97 = FP8_CONVS[97]['chunks']
            wms97 = [w_sb[(97, ci_)].rearrange(
                "p (d i m) -> p d i m", d=ch['ndj0'], i=2)
                for ci_, ch in enumerate(ch97)]
            JC97 = [(0, 64), (64, 64), (128, 32)]
            n97 = len(ch97)
            F97 = 8 * IMG + 8
            for blk in (range(5) if only in (None, 97) else ()):
                i0 = blk * BLOCK_I
                wins97 = {}
                for ci_, ch in enumerate(ch97):
                    G, Rw, CH, off = ch['G'], ch['Rw'], ch['CH'], ch['off']
                    win = rpool.tile([G * Rw, F97], FP8,
                                     name=f"win97g_{ci_}_{blk}",
                                     tag=f"win97g_{ci_}", bufs=4)
                    # flat window; DR plane pair read as +8B overlap
                    src = bass.AP(
                        x8g97, (i0 + off) * 8 * IMG,
                        [[8 * CH, G], [8 * IMG, Rw], [1, F97]])
                    nc.sync.dma_start(out=win[:], in_=src)
                    wins97[ci_] = win[:]
                ps97 = {}
                for jc, (j0, jl) in enumerate(JC97):
                    ps97[jc] = ppool.tile([128, 8 * jl], F32,
                                          name=f"ps97_{blk}_{jc}", tag="acc")
                for jc, (j0, jl) in enumerate(JC97):
                    for ci_, ch in enumerate(ch97):
                        ndj0 = ch['ndj0']
                        a = wins97[ci_]
                        for dj0 in range(ndj0):
                            lhsT = wms97[ci_][:, dj0, :, 0:128]
                            rhs = bass.AP(
                                a.tensor,
                                a.offset + 16 * dj0 + 8 * j0,
                                [list(a.ap[0]), [8, 2], [1, 8 * jl]])
                            nc.tensor.matmul(
                                ps97[jc][:], lhsT, rhs,
                                start=(ci_ == 0 and dj0 == 0),
                                stop=(ci_ == n97 - 1 and dj0 == ndj0 - 1),
                                perf_mode=DR)
                if blk >= 1 and defer25:
                    _fp8_win(*defer25.pop(0))
                for jc, (j0, jl) in enumerate(JC97):
                    for b in range(8):
                        scr = spool.tile([128, jl], F32,
                                         name=f"sq97_{blk}_{jc}_{b}",
                                         tag="scr")
                        col = col_base[(97,)] + b * 15 + blk * 3 + jc
                        nc.scalar.activation(
                            out=scr[:], in_=ps97[jc][:, b::8], func=SQ,
                            scale=act97,
                            accum_out=stage[:, col:col + 1])

            # --- fp8-DR conv K=193: 2 blocks x 8 group images, blk-outer ---
            S = _S(193)
            act193 = float(np.sqrt(1.0 / (float(S) ** 2 * K193_SCALE)))
            w193r = [w193[ci_].rearrange("p (d i m) -> p d i m",
                                         d=ch['CH'], i=2)
                     for ci_, ch in enumerate(W193_CHUNKS)]
            for blk in (range(2) if only in (None, 193) else ()):
                i0 = blk * BLOCK_I
                wins193 = {}
                for ci_, ch in enumerate(W193_CHUNKS):
                    G, Rw, CH, off = ch['G'], ch['Rw'], ch['CH'], ch['off']
                    F = 8 * (CH - 1) + 8 * S
                    win = rpool.tile([G * Rw, 2 * F], FP8,
                                     name=f"win193_{ci_}_{blk}",
                                     tag=f"win193_{ci_}", bufs=3)
                    for i_ in range(2):
                        src = bass.AP(
                            x8g,
                            (i0 + off + Rw * i_) * 8 * IMG,
                            [[8 * CH, G], [8 * IMG, Rw], [1, F]])
                        nc.sync.dma_start(
                            out=win[:, i_ * F:(i_ + 1) * F], in_=src)
                    wins193[ci_] = win.rearrange("p (i f) -> p i f", i=2)
                ps193 = ppool.tile([128, 8 * S], F32,
                                   name=f"ps193_{blk}", tag="acc")
                for ci_, ch in enumerate(W193_CHUNKS):
                    CH = ch['CH']
                    for dj0 in range(CH):
                        nc.tensor.matmul(
                            ps193[:], w193r[ci_][:, dj0, :, :],
                            wins193[ci_][:, :, 8 * dj0:8 * dj0 + 8 * S],
                            start=(ci_ == 0 and dj0 == 0),
                            stop=(ci_ == len(W193_CHUNKS) - 1
                                  and dj0 == CH - 1),
                            perf_mode=DR)
                if blk == 0 and only is None:
                    while defer25:
                        _fp8_win(*defer25.pop(0))
                    # 25/49 + 97 stage cols complete: fold+reduce+out here
                    # so only conv193's own post-processing tails the iter
                    post_2549()
                    post_97()
                for b in range(8):
                    scr = spool.tile([128, S], F32, name=f"sq193_{blk}_{b}",
                                     tag="scr")
                    col = col_base[(193,)] + b * 2 + blk
                    nc.scalar.activation(
                        out=scr[:], in_=ps193[:, b::8], func=SQ,
                        scale=act193,
                        accum_out=stage[:, col:col + 1])
            if only is None:
                post_193()
            if rep is not None:
                rep.__exit__(None, None, None)
    return nc


def _coalesce_pe_sem_incs(nc):
    """Drop per-matmul PE semaphore increments except on stop_tensor_calc
    matmuls, remapping every wait to the kept-increment count.

    Matmuls complete in program order, and (asserted below) every wait value
    on a matmul-produced semaphore lands exactly on a stop matmul, so the
    remapped wait fires at the completion of the same instruction. This
    removes ~1650 serialized EVT_SEM writes (~26ns each) from the PE stream.
    """
    f = nc.m.functions[0]
    # collect per-semaphore update lists (program order within each block)
    upd = {}
    blk_of = {}
    nonmm = set()
    waits_on = {}
    for bi, bb in enumerate(f.blocks):
        for inst in bb.instructions:
            si = inst.sync_info
            if si is None:
                continue
            for u in (si.on_update or []):
                if u.sync_type != 'semaphore':
                    continue
                if type(inst).__name__ == 'InstMatmult':
                    upd.setdefault(u.id, []).append(
                        (inst, bool(inst.stop_tensor_calc), u))
                    if u.id in blk_of and blk_of[u.id] != bi:
                        nonmm.add(u.id)
                    blk_of[u.id] = bi
                else:
                    nonmm.add(u.id)
            for w in (si.on_wait or []):
                if w.sync_type == 'semaphore':
                    waits_on.setdefault(w.id, []).append((inst, w))
    stripped = remapped = 0
    for sem_id, ups in upd.items():
        if sem_id in nonmm:
            continue
        ws = waits_on.get(sem_id, [])
        ok = all(u.update_mode == 'sem-inc' and u.update_value == 1
                 and u.update_reg is None for _, _, u in ups)
        pref = [0]
        for _, st, _ in ups:
            pref.append(pref[-1] + (1 if st else 0))
        nstops = pref[-1]

        def _new_wait(V):
            # stops-count to wait for: the stop at-or-after update V
            # (waiting on a later instruction is always safe)
            return pref[V] if ups[V - 1][1] else pref[V] + 1

        for _, w in ws:
            V = w.wait_value
            if (w.wait_mode != 'sem-ge-imm' or w.wait_reg is not None
                    or V is None or V < 1 or V > len(ups)
                    or _new_wait(V) > nstops):
                ok = False
        if not ok:
            continue
        for winst, w in ws:
            si = winst.sync_info
            new_waits = []
            for ow in si.on_wait:
                if ow.sync_type == 'semaphore' and ow.id == sem_id:
                    new_waits.append(mybir.SyncWait(
                        sync_type=ow.sync_type, id=ow.id,
                        ant_name=ow.ant_name, wait_mode=ow.wait_mode,
                        wait_value=_new_wait(ow.wait_value),
                        wait_reg=ow.wait_reg))
                    remapped += 1
                else:
                    new_waits.append(ow)
            winst.sync_info = mybir.SyncInfo(
                on_wait=new_waits, on_update=list(si.on_update or []))
        for inst, st, u in ups:
            if st:
                continue
            si = inst.sync_info
            new_upd = [ou for ou in si.on_update
                       if not (ou.sync_type == 'semaphore'
                               and ou.id == sem_id)]
            inst.sync_info = mybir.SyncInfo(
                on_wait=list(si.on_wait or []), on_update=new_upd)
            stripped += 1
    return stripped, remapped


_NC_CACHE = {}


def _get_nc(repeat=1):
    if repeat not in _NC_CACHE:
        nc = _build_nc(repeat=repeat)
        _coalesce_pe_sem_incs(nc)
        nc.compile()
        _NC_CACHE[repeat] = nc
    return _NC_CACHE[repeat]


def kernel(x, w0, w1, w2, w3):
    in_maps = build_in_maps(x, w0, w1, w2, w3)
    nc = _get_nc()
    r = run_bass_kernel_spmd(nc, in_maps, list(range(NCORES)))
    final = np.concatenate([np.asarray(r.results[c]["out"], dtype=np.float32)
                            for c in range(NCORES)], axis=0)
    for g in range(2):
        p = sum(np.asarray(r.results[4 * g + j]["out193"], dtype=np.float32)
                for j in range(4))
        final[8 * g:8 * g + 8, :, 3] = p
        p = sum(np.asarray(r.results[4 * g + j]["out97"], dtype=np.float32)
                for j in range(4))
        final[8 * g:8 * g + 8, :, 2] = p
    return final


# revision 30
# speedup vs baseline: 1.0276x; 1.0174x over previous
"""Trainium2 Bass kernel for nn_Net_34763465294339.

Four single-channel VALID convs (K=25/49/97/193, 16 output channels each) on
x[16,1,256,256], each squared + spatially averaged / scale -> stack -> fold
16 channels into 8 by adding halves. Output [16,8,4] f32.

Sharding: data-parallel over batch, 2 images per core, weights replicated.

Resident-window conv (v3): x rows stay in DRAM in dense layout; per
output-row block a [rows, planes, cols] window tile is DMA'd with large
contiguous per-partition runs (no im2col gather). Kernel-column shifts are
expressed as overlapping column offsets in the matmul rhs AP; kernel-row
shifts live in zero-padded stationary weights (contraction over window rows).

  K=25/49/97 run in fp8e4 with perf_mode=DoubleRow: contraction packs
  (g-replica, row) on partitions x 2 interleave planes, giving 2G kernel
  columns (dj) per matmul. Window planes are pre-shifted by one column so the
  dj pair comes from the plane dim. Per-block/dj0 weights are AP slices of
  one padded matrix per conv ([(g,r), dj0, i, (u,o)] with u = 8*t + s for
  multi-block windows). The 1/(S^2*scale) factor is applied as the
  activation pre-scale (fp8 weights cannot be pre-scaled: underflow).

  v3 structure (measured PE stream rate ~0.45-0.54 ns/output-row state-
  dependent, Ldweights fully hidden -> bound by total streamed psum rows,
  613.5k/core, with HWDGE DMA-issue pacing the second-order cost):
  - windows are FLAT (no duplicated pre-shifted planes): the DR plane pair
    is expressed as an overlapping +2B (+8B for the 8-img group layout)
    stride in the matmul rhs AP, halving window DMA bytes/descriptors; one
    3D-AP DMA covers all G replicas of a chunk (per plane for conv193).
  - conv25 repacked NBW=1 (G=4, Rw=32, CH=8, ndj0=4): 4 MMs/block, 53.8k
    rows vs 69.6k. conv49 stays NBW=2 single-chunk (13 MMs/block): the
    denser 4-chunk 12 MM/block packing loses more to HWDGE issue time
    than it saves in PE rows. conv97 uses 6 row-chunks {18x5,14} with
    G=7/9: 41 MMs/block vs 42.
  - conv25/49 run window-outer (window's matmuls then its Squares) so
    psum-bank rotation pipelines acts under the next window's matmuls.
  - matmul loops keep ONE psum accumulation region active at a time
    (conv49 t-outer, conv97 jc-outer): alternating psum targets
    mid-accumulation measurably costs ~10% of PE stream rate.
  - conv193 runs block-outer so blk0's Squares hide under blk1's matmuls.
  - conv49/conv25 windows emit interleaved (13:25) so the two independent
    DMA queues (sync HWDGE / Pool SWDGE) face evenly-paced demand; the
    last 5 conv25 windows + last conv49 window are held back as PE
    filler at the conv97 block boundaries and the conv193 blk0->blk1
    boundary (measured: the 97->193 transition itself does not stall).
  - fold/reduce/output are split per conv phase and interleaved into later
    convs' matmul streams (25/49+97 folds between the conv193 blocks);
    output DMAs ride the Act HWDGE queue so the sync window queue is
    never head-blocked across iterations.

  K=193 also runs fp8-DR with the pair planes carrying row-halves
  (contraction pairs (p, p+100) cover all 200 window rows -> ONE matmul per
  dj; post-fold fp8 error 1.72% vs the 2e-2 gate, deterministic inputs).
  It is also resharded: quads of cores share an 8-image group, each core
  runs the SAME program blocks {0,1} on x8g whose content is row-shifted
  by 16*(core%4) at upload, so N = 8 img * 64 = 512 and each core emits
  per-image partial energies (out193) that the host sums across the quad.
  conv97 is likewise resharded (x8g97, 5 program blocks, j-chunks).
"""
import numpy as np
import ml_dtypes

import concourse.bass as bass
import concourse.bacc as bacc
import concourse.mybir as mybir
from concourse.tile import TileContext
from concourse.bass_utils import run_bass_kernel_spmd

BF16 = mybir.dt.bfloat16
FP8 = mybir.dt.float8e4
F32 = mybir.dt.float32
NP_FP8 = ml_dtypes.float8_e4m3
NP_BF16 = ml_dtypes.bfloat16

IMG = 256
X8ROWS = 292  # padded rows for window reads past image end
NCORES = 8
BLOCK_I = 8

# fp8 convs: K -> (NBW, scale, chunks); each chunk is a row-range of the
# contraction packed as (G, Rw, CH, ndj0, off): window rows off..off+Rw-1,
# G column-replicas, dj = CH*g + 2*dj0 + i. conv97 splits its 104-row span
# into two chunks so dj-packing rises from 2/MM to 4-6/MM.
FP8_CONVS = {
    25: dict(NBW=1, scale=1.0,
             chunks=[dict(G=4, Rw=32, CH=8, ndj0=4, off=0)]),
    49: dict(NBW=2, scale=2.0,
             chunks=[dict(G=2, Rw=64, CH=26, ndj0=13, off=0)]),
    97: dict(NBW=1, scale=4.0,
             chunks=[dict(G=7, Rw=18, CH=14, ndj0=7, off=o)
                     for o in (0, 18, 36, 54)]
             + [dict(G=4, Rw=32, CH=26, ndj0=13, off=72)]),
}
# processing order: conv49 first so conv25's window DMAs (own queue) run
# ahead during conv49's PE time
FP8_ORDER = (49, 25)
WIN_BUFS = {25: 6, 49: 8}
K193_SCALE = 8.0
CONVS = [25, 49, 97, 193]


def _S(K):
    return IMG - K + 1


def _F(K, ch):
    """fp8 flat window free size: max rhs read 4*(ndj0-1) + 2S + 2 (the DR
    plane pair is an overlapping +2B shift expressed in the rhs AP)."""
    f = 4 * (ch['ndj0'] - 1) + 2 * _S(K) + 2
    return (f + 15) // 16 * 16


def build_fp8_w(w, K, ch, NBW):
    """w: [16,K,K] f32 raw. Returns [G*Rw, ndj0*2*U*16] fp8 where
    M[(g,r), dj0, i, (u,o)] = w[o, off+r-u, CH*g+2*dj0+i], U = 8*NBW."""
    G, Rw, CH, ndj0, off = ch['G'], ch['Rw'], ch['CH'], ch['ndj0'], ch['off']
    U = 8 * NBW
    M = np.zeros((G, Rw, ndj0, 2, U, 16), np.float32)
    r = np.arange(Rw)
    for g in range(G):
        for dj0 in range(ndj0):
            for i in range(2):
                dj = CH * g + 2 * dj0 + i
                if dj >= K:
                    continue
                for u in range(U):
                    d = off + r - u
                    valid = (d >= 0) & (d < K)
                    M[g, r[valid], dj0, i, u, :] = w[:, d[valid], dj].T
    return M.reshape(G * Rw, ndj0 * 2 * U * 16).astype(NP_FP8)


# conv193 contraction chunks: rows off..off+2*Rw-1 as pairs (r, r+Rw),
# G dj-replicas, dj = CH*g + dj0
W193_CHUNKS = [dict(G=7, Rw=18, CH=28, off=0),
               dict(G=7, Rw=18, CH=28, off=36),
               dict(G=2, Rw=64, CH=97, off=72)]


def build_w193(w, ch):
    """w: [16,193,193] f32 RAW. Returns [G*Rw, CH*2*128] fp8 with
    M[(g,r), dj0, i, (s,o)] = w[o, off + r + Rw*i - s, CH*g + dj0]."""
    G, Rw, CH, off = ch['G'], ch['Rw'], ch['CH'], ch['off']
    M = np.zeros((G, Rw, CH, 2, 8, 16), np.float32)
    r = np.arange(Rw)
    for g in range(G):
        for s in range(8):
            for i in range(2):
                d = off + r + Rw * i - s
                v = (d >= 0) & (d < 193)
                for dj0 in range(CH):
                    dj = CH * g + dj0
                    if dj >= 193:
                        continue
                    M[g, r[v], dj0, i, s, :] = w[:, d[v], dj].T
    return M.reshape(G * Rw, CH * 2 * 128).astype(NP_FP8)


def _build_fold():
    F = np.zeros((128, 8), dtype=np.float32)
    for p in range(128):
        F[p, (p % 16) % 8] = 1.0
    return F


def _col_layout():
    """fp8 convs: (K, b) -> base col, width nb. conv97: (97,) -> base col,
    then col = base + img*15 + blk*3 + jc. conv193: (193,) -> base col,
    col = base + img*2 + blk."""
    col_base = {}
    c = 0
    for K in (25, 49):
        nb = _S(K) // BLOCK_I
        for b in range(2):
            col_base[(K, b)] = c
            c += nb
    c2549 = c
    col_base[(97,)] = c
    c += 8 * 15  # img*15 + blk*3 + jc
    c97 = c
    col_base[(193,)] = c
    c += 16
    return col_base, c, c2549, c97


def build_in_maps(x, w0, w1, w2, w3):
    """Full inputs -> per-core input dicts for the compiled nc."""
    x = np.asarray(x, dtype=np.float32).reshape(16, IMG, IMG)
    ws = {25: w0, 49: w1, 97: w2, 193: w3}

    shared = {}
    for K in (25, 49, 97):
        w = np.asarray(ws[K], dtype=np.float32).reshape(16, K, K)
        for ci_, ch in enumerate(FP8_CONVS[K]['chunks']):
            shared[f"w{K}_{ci_}"] = build_fp8_w(w, K, ch,
                                                FP8_CONVS[K]['NBW'])
    w = np.asarray(ws[193], dtype=np.float32).reshape(16, 193, 193)
    for ci_, ch in enumerate(W193_CHUNKS):
        shared[f"w193_{ci_}"] = build_w193(w, ch)
    shared["fold"] = _build_fold()

    in_maps = []
    for c in range(NCORES):
        m = dict(shared)
        # [row, (col, b)] interleaved pair of images
        pair = np.ascontiguousarray(
            x[2 * c:2 * c + 2].transpose(1, 2, 0)).reshape(IMG, 2 * IMG)
        x8 = np.zeros((X8ROWS, 2 * IMG), np.float32)
        x8[:IMG] = pair
        m["x8"] = x8.astype(NP_FP8)
        # conv193 group input: 8 images of group c//4, row-shifted by
        # 16*(c%4) so program blocks {0,1} compute real blocks 2*(c%4)+{0,1}
        g = c // 4
        oct_ = np.zeros((IMG + 3, 8 * IMG), np.float32)
        oct_[:IMG] = np.ascontiguousarray(
            x[8 * g:8 * g + 8].transpose(1, 2, 0)).reshape(IMG, 8 * IMG)
        r0 = 16 * (c % 4)
        m["x8g"] = np.ascontiguousarray(
            oct_[r0:r0 + 209]).astype(NP_FP8)
        # conv97 group input: same 8 images, row shift 40*(c%4) so program
        # blocks {0..4} compute real blocks 5*(c%4)+{0..4}
        r0 = 40 * (c % 4)
        m["x8g97"] = np.ascontiguousarray(
            oct_[r0:r0 + 138]).astype(NP_FP8)
        in_maps.append(m)
    return in_maps


QSPLIT49 = True


def _build_nc(repeat=1, only=None):
    nc = bacc.Bacc("TRN2", target_bir_lowering=False)
    x8 = nc.dram_tensor("x8", [X8ROWS, 2 * IMG], FP8, kind="ExternalInput")
    x8g = nc.dram_tensor("x8g", [209, 8 * IMG], FP8, kind="ExternalInput")
    x8g97 = nc.dram_tensor("x8g97", [138, 8 * IMG], FP8,
                           kind="ExternalInput")
    w_h = {}
    for K in (25, 49, 97):
        c = FP8_CONVS[K]
        for ci_, ch in enumerate(c['chunks']):
            w_h[(K, ci_)] = nc.dram_tensor(
                f"w{K}_{ci_}",
                [ch['G'] * ch['Rw'], ch['ndj0'] * 2 * 8 * c['NBW'] * 16],
                FP8, kind="ExternalInput")
    w193_h = {}
    for ci_, ch in enumerate(W193_CHUNKS):
        w193_h[ci_] = nc.dram_tensor(
            f"w193_{ci_}", [ch['G'] * ch['Rw'], ch['CH'] * 2 * 128], FP8,
            kind="ExternalInput")
    fold_h = nc.dram_tensor("fold", [128, 8], F32, kind="ExternalInput")
    out = nc.dram_tensor("out", [2, 8, 4], F32, kind="ExternalOutput")
    out193 = nc.dram_tensor("out193", [8, 8], F32, kind="ExternalOutput")
    out97 = nc.dram_tensor("out97", [8, 8], F32, kind="ExternalOutput")

    col_base, TOT, C2549, C97 = _col_layout()
    SQ = mybir.ActivationFunctionType.Square
    DR = mybir.MatmulPerfMode.DoubleRow

    with TileContext(nc) as tc:
        with tc.tile_pool(name="consts", bufs=1) as cpool, \
             tc.tile_pool(name="winp", bufs=2) as rpool, \
             tc.tile_pool(name="scrp", bufs=4) as spool, \
             tc.tile_pool(name="accp", bufs=8, space="PSUM") as ppool:
            w_sb = {}
            for key, h in w_h.items():
                K, ci_ = key
                t = cpool.tile(list(h.shape), FP8, name=f"w{K}_{ci_}sb",
                               tag=f"w{K}_{ci_}")
                nc.gpsimd.dma_start(out=t[:], in_=h[:])
                w_sb[key] = t
            w193 = {}
            for ci_, ch in enumerate(W193_CHUNKS):
                t = cpool.tile(list(w193_h[ci_].shape), FP8,
                               name=f"w193_{ci_}sb", tag=f"w193_{ci_}")
                nc.gpsimd.dma_start(out=t[:], in_=w193_h[ci_][:])
                w193[ci_] = t
            fold_sb = cpool.tile([128, 8], F32, name="fold_sb", tag="fold")
            nc.sync.dma_start(out=fold_sb[:], in_=fold_h[:])
            stage = cpool.tile([128, TOT], F32, name="stage", tag="stage")

            rep = tc.For_i(0, repeat) if repeat != 1 else None
            if rep is not None:
                rep.__enter__()

            # --- deferred post-processing emitters -------------------------
            res = spool.tile([8, 8], F32, name="res", tag="res", bufs=1)
            res97 = spool.tile([8, 8], F32, name="res97", tag="res97",
                               bufs=1)
            res193 = spool.tile([8, 8], F32, name="res193", tag="res193",
                                bufs=1)

            def post_2549():
                # fold (s,o) partitions -> o%8 over the 25/49 stage range,
                # reduce per (conv, image) col group, DMA out on Act HWDGE
                fps = ppool.tile([8, C2549], F32, name="fold_ps1", tag="acc")
                nc.tensor.matmul(fps[:], fold_sb[:], stage[:, 0:C2549],
                                 start=True, stop=True)
                for ci, K in enumerate((25, 49)):
                    nb = _S(K) // BLOCK_I
                    for b in range(2):
                        c0 = col_base[(K, b)]
                        oc = b * 4 + ci
                        nc.vector.reduce_sum(out=res[:8, oc:oc + 1],
                                             in_=fps[:8, c0:c0 + nb],
                                             axis=mybir.AxisListType.X)
                a = res[:8, :]
                src = bass.AP(a.tensor, a.offset,
                              [list(a.ap[0]), [4, 2], [1, 2]])
                dst = bass.AP(out, 0, [[4, 8], [32, 2], [1, 2]])
                nc.scalar.dma_start(out=dst, in_=src)

            def post_97():
                fps = ppool.tile([8, 120], F32, name="fold_ps97", tag="acc")
                nc.tensor.matmul(fps[:], fold_sb[:],
                                 stage[:, col_base[(97,)]:C97],
                                 start=True, stop=True)
                for i in range(8):
                    nc.vector.reduce_sum(
                        out=res97[:8, i:i + 1],
                        in_=fps[:8, 15 * i:15 * i + 15],
                        axis=mybir.AxisListType.X)
                dst97 = bass.AP(out97, 0, [[1, 8], [8, 8]])
                nc.scalar.dma_start(out=dst97, in_=res97[:8, :])

            def post_193():
                fps = ppool.tile([8, 16], F32, name="fold_ps193", tag="acc")
                nc.tensor.matmul(fps[:], fold_sb[:],
                                 stage[:, col_base[(193,)]:TOT],
                                 start=True, stop=True)
                for i in range(8):
                    nc.vector.reduce_sum(
                        out=res193[:8, i:i + 1],
                        in_=fps[:8, 2 * i:2 * i + 2],
                        axis=mybir.AxisListType.X)
                dst193 = bass.AP(out193, 0, [[1, 8], [8, 8]])
                nc.scalar.dma_start(out=dst193, in_=res193[:8, :])

            # --- fp8 DoubleRow convs 49 + 25, window-outer, interleaved ----
            def _fp8_ctx(K):
                c = FP8_CONVS[K]
                chunks = c['chunks']
                S = _S(K)
                nb = S // BLOCK_I
                return dict(
                    K=K, NBW=c['NBW'], chunks=chunks, S=S, nb=nb,
                    act_scale=float(
                        np.sqrt(1.0 / (float(S) ** 2 * c['scale']))),
                    nwin=(nb + c['NBW'] - 1) // c['NBW'],
                    dma_eng=nc.gpsimd if K == 25 else nc.sync,
                    wms=[w_sb[(K, ci_)].rearrange(
                        "p (d i m) -> p d i m", d=ch['ndj0'], i=2)
                        for ci_, ch in enumerate(chunks)])

            def _fp8_win(ctx, wi):
                K, NBW, chunks, S = (ctx['K'], ctx['NBW'], ctx['chunks'],
                                     ctx['S'])
                nchunks = len(chunks)
                i0 = wi * NBW * BLOCK_I
                nts = min(NBW, ctx['nb'] - wi * NBW)
                wins = {}
                for ci_, ch in enumerate(chunks):
                    G, Rw, CH = ch['G'], ch['Rw'], ch['CH']
                    F = _F(K, ch)
                    win = rpool.tile([G * Rw, F], FP8,
                                     name=f"win{K}_{ci_}_{wi}",
                                     tag=f"win{K}_{ci_}",
                                     bufs=WIN_BUFS[K])
                    # one flat 3D-AP DMA covering all G replicas; the
                    # DR plane pair is read later as a +2B overlap
                    src = bass.AP(
                        x8, (i0 + ch['off']) * 2 * IMG,
                        [[2 * CH, G], [2 * IMG, Rw], [1, F]])
                    ctx['dma_eng'].dma_start(out=win[:], in_=src)
                    wins[ci_] = win[:]
                psums = {}
                for t in range(nts):
                    psums[t] = ppool.tile([128, 2 * S], F32,
                                          name=f"ps{K}_{wi}_{t}",
                                          tag="acc")
                for t in range(nts):
                    for ci_, ch in enumerate(chunks):
                        ndj0 = ch['ndj0']
                        a = wins[ci_]
                        for dj0 in range(ndj0):
                            rhs = bass.AP(
                                a.tensor, a.offset + 4 * dj0,
                                [list(a.ap[0]), [2, 2], [1, 2 * S]])
                            lhsT = ctx['wms'][ci_][
                                :, dj0, :, 8 * t * 16:8 * t * 16 + 128]
                            nc.tensor.matmul(
                                psums[t][:], lhsT, rhs,
                                start=(ci_ == 0 and dj0 == 0),
                                stop=(ci_ == nchunks - 1
                                      and dj0 == ndj0 - 1),
                                perf_mode=DR)
                for t in range(nts):
                    blk = wi * NBW + t
                    for b in range(2):
                        scr = spool.tile([128, S], F32,
                                         name=f"sq{K}_{blk}_{b}",
                                         tag="scr")
                        col = col_base[(K, b)] + blk
                        nc.scalar.activation(
                            out=scr[:], in_=psums[t][:, b::2],
                            func=SQ, scale=ctx['act_scale'],
                            accum_out=stage[:, col:col + 1])

            ks = [k for k in FP8_ORDER if only in (None, k)]
            defer25 = []
            if len(ks) == 2:
                cA, cB = _fp8_ctx(49), _fp8_ctx(25)
                # hold 5 conv25 windows + the last conv49 window back as PE
                # filler at the conv97 block boundaries and the 97->193
                # transition
                nB = cB['nwin'] - 5
                nA = cA['nwin'] - 1
                done_b = 0
                for wiA in range(nA):
                    _fp8_win(cA, wiA)
                    want = (wiA + 1) * nB // nA
                    while done_b < want:
                        _fp8_win(cB, done_b)
                        done_b += 1
                defer25 = ([(cB, wi) for wi in range(nB, cB['nwin'])]
                           + [(cA, nA)])
            else:
                for K in ks:
                    ctx = _fp8_ctx(K)
                    for wi in range(ctx['nwin']):
                        _fp8_win(ctx, wi)

            # --- fp8-DR conv K=97, resharded: 5 blocks x 8 group images ----
            S97 = _S(97)
            act97 = float(np.sqrt(1.0 / (float(S97) ** 2 * 4.0)))
            ch97 = FP8_CONVS[97]['chunks']
            wms97 = [w_sb[(97, ci_)].rearrange(
                "p (d i m) -> p d i m", d=ch['ndj0'], i=2)
                for ci_, ch in enumerate(ch97)]
            JC97 = [(0, 64), (64, 64), (128, 32)]
            n97 = len(ch97)
            F97 = 8 * IMG + 8
            for blk in (range(5) if only in (None, 97) else ()):
                i0 = blk * BLOCK_I
                wins97 = {}
                for ci_, ch in enumerate(ch97):
                    G, Rw, CH, off = ch['G'], ch['Rw'], ch['CH'], ch['off']
                    win = rpool.tile([G * Rw, F97], FP8,
                                     name=f"win97g_{ci_}_{blk}",
                                     tag=f"win97g_{ci_}", bufs=4)
                    # flat window; DR plane pair read as +8B overlap
                    src = bass.AP(
                        x8g97, (i0 + off) * 8 * IMG,
                        [[8 * CH, G], [8 * IMG, Rw], [1, F97]])
                    nc.sync.dma_start(out=win[:], in_=src)
                    wins97[ci_] = win[:]
                ps97 = {}
                for jc, (j0, jl) in enumerate(JC97):
                    ps97[jc] = ppool.tile([128, 8 * jl], F32,
                                          name=f"ps97_{blk}_{jc}", tag="acc")
                for jc, (j0, jl) in enumerate(JC97):
                    for ci_, ch in enumerate(ch97):
                        ndj0 = ch['ndj0']
                        a = wins97[ci_]
                        for dj0 in range(ndj0):
                            lhsT = wms97[ci_][:, dj0, :, 0:128]
                            rhs = bass.AP(
                                a.tensor,
                                a.offset + 16 * dj0 + 8 * j0,
                                [list(a.ap[0]), [8, 2], [1, 8 * jl]])
                            nc.tensor.matmul(
                                ps97[jc][:], lhsT, rhs,
                                start=(ci_ == 0 and dj0 == 0),
                                stop=(ci_ == n97 - 1 and dj0 == ndj0 - 1),
                                perf_mode=DR)
                if blk >= 1 and defer25:
                    _fp8_win(*defer25.pop(0))
                for jc, (j0, jl) in enumerate(JC97):
                    for b in range(8):
                        scr = spool.tile([128, jl], F32,
                                         name=f"sq97_{blk}_{jc}_{b}",
                                         tag="scr")
                        col = col_base[(97,)] + b * 15 + blk * 3 + jc
                        nc.scalar.activation(
                            out=scr[:], in_=ps97[jc][:, b::8], func=SQ,
                            scale=act97,
                            accum_out=stage[:, col:col + 1])

            # --- fp8-DR conv K=193: 2 blocks x 8 group images, blk-outer ---
            S = _S(193)
            act193 = float(np.sqrt(1.0 / (float(S) ** 2 * K193_SCALE)))
            w193r = [w193[ci_].rearrange("p (d i m) -> p d i m",
                                         d=ch['CH'], i=2)
                     for ci_, ch in enumerate(W193_CHUNKS)]
            for blk in (range(2) if only in (None, 193) else ()):
                i0 = blk * BLOCK_I
                wins193 = {}
                for ci_, ch in enumerate(W193_CHUNKS):
                    G, Rw, CH, off = ch['G'], ch['Rw'], ch['CH'], ch['off']
                    F = 8 * (CH - 1) + 8 * S
                    win = rpool.tile([G * Rw, 2 * F], FP8,
                                     name=f"win193_{ci_}_{blk}",
                                     tag=f"win193_{ci_}", bufs=3)
                    for i_ in range(2):
                        src = bass.AP(
                            x8g,
                            (i0 + off + Rw * i_) * 8 * IMG,
                            [[8 * CH, G], [8 * IMG, Rw], [1, F]])
                        nc.sync.dma_start(
                            out=win[:, i_ * F:(i_ + 1) * F], in_=src)
                    wins193[ci_] = win.rearrange("p (i f) -> p i f", i=2)
                ps193 = ppool.tile([128, 8 * S], F32,
                                   name=f"ps193_{blk}", tag="acc")
                for ci_, ch in enumerate(W193_CHUNKS):
                    CH = ch['CH']
                    for dj0 in range(CH):
                        nc.tensor.matmul(
                            ps193[:], w193r[ci_][:, dj0, :, :],
                            wins193[ci_][:, :, 8 * dj0:8 * dj0 + 8 * S],
                            start=(ci_ == 0 and dj0 == 0),
                            stop=(ci_ == len(W193_CHUNKS) - 1
                                  and dj0 == CH - 1),
                            perf_mode=DR)
                if blk == 0 and only is None:
                    while defer25:
                        _fp8_win(*defer25.pop(0))
                    # 25/49 + 97 stage cols complete: fold+reduce+out here
                    # so only conv193's own post-processing tails the iter
                    post_2549()
                    post_97()
                for b in range(8):
                    scr = spool.tile([128, S], F32, name=f"sq193_{blk}_{b}",
                                     tag="scr")
                    col = col_base[(193,)] + b * 2 + blk
                    nc.scalar.activation(
                        out=scr[:], in_=ps193[:, b::8], func=SQ,
                        scale=act193,
                        accum_out=stage[:, col:col + 1])
            if only is None:
                post_193()
            if rep is not None:
                rep.__exit__(None, None, None)
    return nc


def _coalesce_pe_sem_incs(nc):
    """Drop per-matmul PE semaphore increments except on stop_tensor_calc
    matmuls, remapping every wait to the kept-increment count.

    Matmuls complete in program order, and (asserted below) every wait value
    on a matmul-produced semaphore lands exactly on a stop matmul, so the
    remapped wait fires at the completion of the same instruction. This
    removes ~1650 serialized EVT_SEM writes (~26ns each) from the PE stream.
    """
    f = nc.m.functions[0]
    # collect per-semaphore update lists (program order within each block)
    upd = {}
    blk_of = {}
    nonmm = set()
    waits_on = {}
    for bi, bb in enumerate(f.blocks):
        for inst in bb.instructions:
            si = inst.sync_info
            if si is None:
                continue
            for u in (si.on_update or []):
                if u.sync_type != 'semaphore':
                    continue
                if type(inst).__name__ == 'InstMatmult':
                    upd.setdefault(u.id, []).append(
                        (inst, bool(inst.stop_tensor_calc), u))
                    if u.id in blk_of and blk_of[u.id] != bi:
                        nonmm.add(u.id)
                    blk_of[u.id] = bi
                else:
                    nonmm.add(u.id)
            for w in (si.on_wait or []):
                if w.sync_type == 'semaphore':
                    waits_on.setdefault(w.id, []).append((inst, w))
    stripped = remapped = 0
    for sem_id, ups in upd.items():
        if sem_id in nonmm:
            continue
        ws = waits_on.get(sem_id, [])
        ok = all(u.update_mode == 'sem-inc' and u.update_value == 1
                 and u.update_reg is None for _, _, u in ups)
        pref = [0]
        for _, st, _ in ups:
            pref.append(pref[-1] + (1 if st else 0))
        nstops = pref[-1]

        def _new_wait(V):
            # stops-count to wait for: the stop at-or-after update V
            # (waiting on a later instruction is always safe)
            return pref[V] if ups[V - 1][1] else pref[V] + 1

        for _, w in ws:
            V = w.wait_value
            if (w.wait_mode != 'sem-ge-imm' or w.wait_reg is not None
                    or V is None or V < 1 or V > len(ups)
                    or _new_wait(V) > nstops):
                ok = False
        if not ok:
            continue
        for winst, w in ws:
            si = winst.sync_info
            new_waits = []
            for ow in si.on_wait:
                if ow.sync_type == 'semaphore' and ow.id == sem_id:
                    new_waits.append(mybir.SyncWait(
                        sync_type=ow.sync_type, id=ow.id,
                        ant_name=ow.ant_name, wait_mode=ow.wait_mode,
                        wait_value=_new_wait(ow.wait_value),
                        wait_reg=ow.wait_reg))
                    remapped += 1
                else:
                    new_waits.append(ow)
            winst.sync_info = mybir.SyncInfo(
                on_wait=new_waits, on_update=list(si.on_update or []))
        for inst, st, u in ups:
            if st:
                continue
            si = inst.sync_info
            new_upd = [ou for ou in si.on_update
                       if not (ou.sync_type == 'semaphore'
                               and ou.id == sem_id)]
            inst.sync_info = mybir.SyncInfo(
                on_wait=list(si.on_wait or []), on_update=new_upd)
            stripped += 1
    return stripped, remapped


_NC_CACHE = {}


def _get_nc(repeat=1):
    if repeat not in _NC_CACHE:
        nc = _build_nc(repeat=repeat)
        _coalesce_pe_sem_incs(nc)
        nc.compile()
        _NC_CACHE[repeat] = nc
    return _NC_CACHE[repeat]


def kernel(x, w0, w1, w2, w3):
    in_maps = build_in_maps(x, w0, w1, w2, w3)
    nc = _get_nc()
    r = run_bass_kernel_spmd(nc, in_maps, list(range(NCORES)))
    final = np.concatenate([np.asarray(r.results[c]["out"], dtype=np.float32)
                            for c in range(NCORES)], axis=0)
    for g in range(2):
        p = sum(np.asarray(r.results[4 * g + j]["out193"], dtype=np.float32)
                for j in range(4))
        final[8 * g:8 * g + 8, :, 3] = p
        p = sum(np.asarray(r.results[4 * g + j]["out97"], dtype=np.float32)
                for j in range(4))
        final[8 * g:8 * g + 8, :, 2] = p
    return final
